# revision 29
# baseline (speedup 1.0000x reference)
"""Contrastive cosine-similarity softmax-CE loss on 8 trn2 NeuronCores.

reference math:
    n1 = f1 / max(||f1||, eps);  n2 = f2 / max(||f2||, eps)
    logits = (n1 @ n2.T) / TEMP                      # [8192, 8192]
    loss = mean_i( logsumexp_j(logits[i, :]) - logits[i, i] )

sharding: f1 rows data-parallel across 8 cores (1024 rows each); f2
(normalized form, per the sharding hint) replicated to every core.
Per-core output is the vector of per-row (lse - l_ii); host averages.

Device-side algorithm per core (all SPMD-uniform, no collectives):
  - both operands are normalized, prescaled by SC=32 and cast to
    fp8e4m3 on the host ("all-gather f2 (or its normalized form)"),
    so the device runs a pure GEMM+exp pipeline: no on-device norms,
    no sum-of-squares matmuls, no Ln/Exp normalize chains.  This keeps
    the PE stream dense, which matters because the PE clock ramps
    (0.65 -> 1.2 -> 2.4 GHz) with continuous busy time.
  - logits are never max-subtracted: |logit| <= 1/0.07 = 14.3 by
    Cauchy-Schwarz, so exp() stays in fp32 range.  Single-pass softmax.
  - main GEMM in fp8e4m3 perf_mode=DoubleRow (2 k-chunks per pass).
    PSUM groups of [128, 2048] (4 banks, double-buffered = all 8
    banks): 12 matmuls fill a group, then one fused ScalarE
    Exp+row-sum drains it (accum_out -> per-row partial softmax
    denominator).  ACT per group (~2.2us) < PE per group (~2.6us), so
    the PE never waits on the drain.
  - the f2 pair order is PERMUTED per core so that the pair holding
    this core's diagonal block comes first: core c streams pairs
    [c, others...].  The diagonal logit of row-block m then sits at
    column m*128+p of the first [128,2048] psum group of row-block m —
    core-invariant — and is extracted by an identity-mask dot on the
    (otherwise idle) DVE straight out of PSUM (fp32, no exp/ln
    roundtrip).  Row sums are permutation-invariant.  This removes the
    bf16 diag operands (3.1MB of DMA) from the end-of-kernel critical
    path entirely.
  - ~32 warmup matmuls on memset scratch run during the DMA fill so
    the PE clock is fully ramped when the first real matmul issues.
  - within a group the pair-half loop is outermost, so the first 6
    matmuls depend only on stage 2s (stage 2s+1 may still be in
    flight).
  - DMA rings: first-needed tensors split across all three rings
    (sync HWDGE / gpsimd SWDGE / scalar HWDGE) for fill bandwidth.
  - _split_excess_waits(): Tile attaches more sync waits per
    instruction than the 64B TPB encodings can carry and walrus
    hard-fails; excess waits are hoisted into standalone
    InstEventSemaphore instructions on the same engine.
"""

import sys

for _p in ("/opt/trn_rl_repo",):
    if _p not in sys.path:
        sys.path.insert(0, _p)

from contextlib import ExitStack

import ml_dtypes
import numpy as np

import concourse.bass as bass
import concourse.tile as tile
from concourse import mybir

FP32 = mybir.dt.float32
BF16 = mybir.dt.bfloat16
FP8 = mybir.dt.float8e4
AF = mybir.ActivationFunctionType
ALU = mybir.AluOpType
AX = mybir.AxisListType

N = 8192        # rows of f1/f2
D = 768         # feature dim
NCORES = 8
MC = N // NCORES        # f1 rows per core (1024)
KT = D // 128           # contraction k-chunks (6)
MT = MC // 128          # f1 row tiles per core (8)
PAIR = 1024             # f2 rows per staged tile
NPAIR = N // PAIR       # 8
NSUP = NPAIR // 2       # supersteps: 2 pairs -> one [128,2048] psum group
GRP = 2 * PAIR          # psum group columns (4 PSUM banks)
TEMP = 0.07
SC = 32.0               # power-of-2 prescale for the fp8 operands
ESCALE = float(1.0 / (SC * SC * TEMP))
NWARM = 30              # PE clock-warmup matmuls during the DMA fill

_WAIT_SPLIT_SKIP = (
    "InstEventSemaphore",
    "InstHalt",
)


def _split_excess_waits(nc: bass.Bass, cap: int = 1) -> None:
    """Hoist per-instruction sync waits beyond `cap` into standalone
    InstEventSemaphore instructions on the same engine."""
    n = 0
    for bb in nc.main_func.blocks:
        new_list = []
        for inst in bb.instructions:
            si = inst.sync_info
            ow = list(si.on_wait) if si is not None and si.on_wait else []
            if len(ow) > cap and type(inst).__name__ not in _WAIT_SPLIT_SKIP:
                excess, keep = ow[:-cap], ow[-cap:]
                for w in excess:
                    n += 1
                    ev = mybir.InstEventSemaphore(
                        name=f"I-waitsplit-{n}",
                        engine=inst.engine,
                        ins=[],
                        outs=[],
                        sync_info=mybir.SyncInfo(on_wait=[w], on_update=[]),
                    )
                    nc.register_instruction(ev)
                    new_list.append(ev)
                si.on_wait = keep
            new_list.append(inst)
        bb.instructions[:] = new_list


def build_program() -> bass.Bass:
    nc = bass.Bass()
    # w8[p, m, k, i] = (n1[m*128+i, k*128+p] * SC) as fp8 — stationary operand
    w8d = nc.declare_dram_parameter("w8", [128, MT, KT, 128], FP8, isOutput=False)
    # x8[q, p, k, n] = (n2[perm[q]*1024+n, k*128+p] * SC) as fp8 — moving
    # operand, pair order permuted per core (diagonal pair first)
    x8d = nc.declare_dram_parameter("x8", [NPAIR, 128, KT, PAIR], FP8, isOutput=False)
    # idm[p, i] = 1.0 if i == p else 0 — diagonal-extraction mask
    idmd = nc.declare_dram_parameter("idm", [128, 128], BF16, isOutput=False)
    out = nc.declare_dram_parameter("out", [128, MT], FP32, isOutput=True)

    with tile.TileContext(nc, pool_alloc_mode="queue") as tc, ExitStack() as ctx:
        singles = ctx.enter_context(tc.tile_pool(name="singles", bufs=1))
        w8 = singles.tile([128, MT, KT, 128], FP8, tag="w8", name="w8")
        spart = singles.tile([128, MT * NSUP], FP32, tag="spart", name="spart")
        pdiag = singles.tile([128, MT], FP32, tag="pdiag", name="pdiag")
        idm = singles.tile([128, 128], BF16, tag="idm", name="idm")
        S = singles.tile([128, MT], FP32, tag="S", name="S")
        lse = singles.tile([128, MT], FP32, tag="lse", name="lse")
        res = singles.tile([128, MT], FP32, tag="res", name="res")
        wsc = singles.tile([128, 2], BF16, tag="wsc", name="wsc")
        xsc = singles.tile([128, 512], BF16, tag="xsc", name="xsc")

        stg = ctx.enter_context(tc.tile_pool(name="stg", bufs=NPAIR))
        stages = [
            stg.tile([128, KT, PAIR], FP8, tag="stage", name="stage")
            for _ in range(NPAIR)
        ]

        # ---- loads: the three DMA rings (sync HWDGE / gpsimd SWDGE /
        # scalar HWDGE) each FIFO their own list; first-needed tensors
        # lead each ring so the fill isn't serialized behind later pairs.
        # The sync HWDGE ring starts fastest (~8us, full HBM burst while
        # alone) — it carries the fill-critical tensors in consumption
        # order.  The gpsimd SWDGE ring (slower start) gets the later
        # stages.  Bulk must NOT ride the scalar ring: its trigger
        # instructions get scheduled behind the first Exp instructions on
        # the Scalar queue (observed firing at ~21us).
        nc.sync.dma_start(w8[:], w8d[:])
        nc.gpsimd.dma_start(stages[0][:], x8d[0])
        nc.scalar.dma_start(idm[:], idmd[:])
        for p in (1, 3, 5):
            nc.sync.dma_start(stages[p][:], x8d[p])
        for p in (2, 4, 6, 7):
            nc.gpsimd.dma_start(stages[p][:], x8d[p])

        nc.any.memset(wsc[:], 0.0)
        nc.any.memset(xsc[:], 0.0)

        # ---- main pipeline: 12 DoubleRow matmuls per [128,2048] psum
        # group, drained by one fused Exp+row-sum on ScalarE.
        with tc.tile_pool(name="pp", bufs=2, space="PSUM") as pp, tc.tile_pool(
            name="ep", bufs=4
        ) as ep:
            # PE clock warmup on scratch data while the fill streams in;
            # the first real matmul overwrites this psum slice (start=True).
            ps_first = pp.tile([128, GRP], FP32, tag="ps", name="ps")
            for _ in range(NWARM):
                nc.tensor.matmul(
                    ps_first[0:2, 0:512],
                    wsc[:],
                    xsc[:],
                    start=True,
                    stop=True,
                    skip_group_check=True,
                )

            def emit_half(ps, s, m, half):
                stx = stages[2 * s + half]
                for j in range(KT // 2):
                    wsl = w8[:, m, 2 * j : 2 * j + 2, :]
                    for h in range(2):
                        c0 = half * PAIR + h * 512
                        nc.tensor.matmul(
                            ps[:, c0 : c0 + 512],
                            wsl,
                            stx[:, 2 * j : 2 * j + 2, h * 512 : (h + 1) * 512],
                            start=(j == 0),
                            stop=(j == KT // 2 - 1),
                            perf_mode=mybir.MatmulPerfMode.DoubleRow,
                        )

            def emit_exp(ps, lo, hi, acc):
                eb = ep.tile([128, GRP], BF16, tag="eb", name="eb")
                nc.scalar.activation(
                    eb[:, lo:hi], ps[:, lo:hi], AF.Exp, scale=ESCALE, accum_out=acc
                )

            def emit_diag(ps, m):
                # diagonal logit of row-block m sits at column m*128+p of
                # this group (pair perm puts the diagonal pair first):
                # identity-mask dot on DVE, straight out of PSUM at fp32.
                jt = ep.tile([128, 128], BF16, tag="jt", name="jt", bufs=2)
                nc.vector.scalar_tensor_tensor(
                    jt[:],
                    ps[:, m * 128 : (m + 1) * 128],
                    1.0,
                    idm[:],
                    op0=ALU.mult,
                    op1=ALU.mult,
                    accum_out=pdiag[:, m : m + 1],
                )

            def col(m, s):
                return slice(m * NSUP + s, m * NSUP + s + 1)

            for s in range(NSUP):
                for m in range(MT):
                    if s == 0 and m == 0:
                        ps = ps_first
                    else:
                        ps = pp.tile([128, GRP], FP32, tag="ps", name="ps")
                    emit_half(ps, s, m, 0)
                    emit_half(ps, s, m, 1)
                    emit_exp(ps, 0, GRP, spart[:, col(m, s)])
                    if s == 0:
                        emit_diag(ps, m)

            # ---- finalize: S = sum of partials; res = ln(S) - ESCALE*pdiag
            nc.vector.reduce_sum(
                S[:], spart[:].rearrange("p (m q) -> p m q", q=NSUP), axis=AX.X
            )
            nc.scalar.activation(lse[:], S[:], AF.Ln)
            nc.vector.scalar_tensor_tensor(
                res[:], pdiag[:], -ESCALE, lse[:], op0=ALU.mult, op1=ALU.add
            )
            nc.sync.dma_start(out[:, :], res[:])

    _split_excess_waits(nc)
    return nc


def make_in_maps(f1: np.ndarray, f2: np.ndarray) -> list[dict[str, np.ndarray]]:
    f1 = np.asarray(f1, dtype=np.float32)
    f2 = np.asarray(f2, dtype=np.float32)
    assert f1.shape == (N, D) and f2.shape == (N, D)
    n1 = f1 / np.maximum(np.linalg.norm(f1, axis=1, keepdims=True), 1e-8)
    n2 = f2 / np.maximum(np.linalg.norm(f2, axis=1, keepdims=True), 1e-8)
    fp8 = ml_dtypes.float8_e4m3
    bf16 = ml_dtypes.bfloat16
    n1s = (n1 * SC).astype(fp8)
    n2s = (n2 * SC).astype(fp8)
    # x8[q, p, k, n] = n2s[q*1024+n, k*128+p]
    x8 = np.ascontiguousarray(n2s.reshape(NPAIR, PAIR, KT, 128).transpose(0, 3, 2, 1))
    idm = np.eye(128, dtype=bf16)
    in_maps = []
    for c in range(NCORES):
        sl = slice(c * MC, (c + 1) * MC)
        # w8[p, m, k, i] = n1s[c*MC + m*128+i, k*128+p]
        w8 = np.ascontiguousarray(
            n1s[sl].reshape(MT, 128, KT, 128).transpose(3, 0, 2, 1)
        )
        # pair order: this core's diagonal pair first, then the rest
        perm = [c] + [q for q in range(NPAIR) if q != c]
        x8c = np.ascontiguousarray(x8[perm])
        in_maps.append({"w8": w8, "x8": x8c, "idm": idm})
    return in_maps


def combine_outputs(outs: list[np.ndarray]) -> np.float32:
    total = 0.0
    for o in outs:
        total += float(np.sum(np.asarray(o, dtype=np.float64)))
    return np.float32(total / float(N))


def run(f1: np.ndarray, f2: np.ndarray, trace: bool = False):
    from concourse.bass_utils import run_bass_kernel_spmd

    nc = build_program()
    in_maps = make_in_maps(f1, f2)
    r = run_bass_kernel_spmd(nc, in_maps, core_ids=list(range(NCORES)), trace=trace)
    outs = [m["out"] for m in r.results]
    return combine_outputs(outs), r


def kernel(f1: np.ndarray, f2: np.ndarray) -> np.ndarray:
    loss, _ = run(f1, f2, trace=False)
    return loss


if __name__ == "__main__":
    f1 = np.random.randn(N, D).astype(np.float32)
    f2 = np.random.randn(N, D).astype(np.float32)
    print(kernel(f1, f2))
